# revision 59
# baseline (speedup 1.0000x reference)
"""Trainium2 Bass kernel for additive (Bahdanau) attention.

reference:
    proj_f = features @ W1_w + W1_b          # [B, L, ATT]
    proj_h = (hidden @ W2_w + W2_b)[:, None] # [B, 1, ATT]
    scores = tanh(proj_f + proj_h) @ V_w + V_b   # [B, L]
    alpha  = softmax(scores, axis=1)
    context = einsum('bl,ble->be', alpha, features)
    returns (alpha, context)

Sharding: data-parallel over batch B=64 across 8 cores (8 examples/core).
Weights replicated. No collectives.

Per-core algorithm (X = 8 examples):
  - host prepack: features cast to bf16, W1 pre-scaled x16 and quantized
    to fp8e4 in DoubleRow (k-pair) layout, W2 cast to bf16.
  - feature tiles loaded TRANSPOSED straight from DRAM via the DMA XBAR
    (128x128 bf16 blocks) into per-example fT [128, ENC-chunk*L] tiles;
    DVE casts them to fp8e4 for the matmul.
  - main matmul in FP8 DoubleRow perf mode (2 k-tiles of 128 per
    instruction): lhsT = W1 fp8 pair chunk, rhs = fT fp8 pairs.
    PSUM [128, 512] f32 accumulates 4 pair-MMs (ENC=1024).
  - ACT applies tanh with scale=1/16 (undoes the W1 scaling) fused with
    per-partition bias = (W1_b + W2_b + hidden @ W2_w) transposed.
  - V-dot on PE in bf16: scores[1, 512] += V_chunk[128,1].T @ tanh_tile,
    trailing the tanh by one block.  (V_b dropped: softmax shift-inv.)
  - softmax per example on DVE/ACT (free-dim reduces on [1, 1024]).
  - context on DVE: scalar_tensor_tensor over the bf16 fT with alpha
    replicated across partitions (gpsimd partition_broadcast); only a
    tiny [128,8] transpose + copy + store sit in the next example's
    PE stream.
"""

import numpy as np

B, L, ENC, DEC, ATT = 64, 1024, 1024, 1024, 1024
N_CORES = 8
X = B // N_CORES  # examples per core
P = 128
NE = ENC // P  # 8
NA = ATT // P  # 8
ND = DEC // P  # 8
NKP = NE // 2  # 4 k-pairs for DoubleRow
LH = 512       # free-dim half for fp32 PSUM bank
NL = L // LH   # 2
W1S = 16.0     # fp8 pre-scale on W1

_CACHE = {}


def _build():
    import concourse.bacc as bacc
    import concourse.mybir as mybir
    import concourse.tile as tile

    f32, bf16 = mybir.dt.float32, mybir.dt.bfloat16
    fp8 = mybir.dt.float8e4
    DR = mybir.MatmulPerfMode.DoubleRow
    Tanh = mybir.ActivationFunctionType.Tanh
    Exp = mybir.ActivationFunctionType.Exp
    mult = mybir.AluOpType.mult
    AX = mybir.AxisListType.X

    nc = bacc.Bacc("TRN2", target_bir_lowering=False, debug=False, num_devices=N_CORES)

    feats = nc.declare_dram_parameter("features", [X, L, ENC], bf16, isOutput=False)
    hid = nc.declare_dram_parameter("hidden_state", [X, DEC], f32, isOutput=False)
    w1q_d = nc.declare_dram_parameter("W1_q", [P, NE * ATT], fp8, isOutput=False)
    w1b = nc.declare_dram_parameter("W1_b", [ATT], f32, isOutput=False)
    w2 = nc.declare_dram_parameter("W2_w", [DEC, ATT], bf16, isOutput=False)
    w2b = nc.declare_dram_parameter("W2_b", [ATT], f32, isOutput=False)
    vw = nc.declare_dram_parameter("V_w", [ATT], f32, isOutput=False)
    alpha_o = nc.declare_dram_parameter("alpha", [X, L], f32, isOutput=True)
    ctx_o = nc.declare_dram_parameter("context", [X, ENC], f32, isOutput=True)

    eye_dram = nc.inline_tensor(np.eye(P, dtype=np.float32), "eye128")

    with tile.TileContext(nc) as tc:
        with (
            tc.tile_pool(name="const", bufs=1) as const,
            tc.tile_pool(name="fT", bufs=4) as ftbp,
            tc.tile_pool(name="f8", bufs=4) as ftp,
            tc.tile_pool(name="mm", bufs=4, space="PSUM") as psum,
            tc.tile_pool(name="sc", bufs=2, space="PSUM") as spsum,
            tc.tile_pool(name="tp", bufs=2, space="PSUM") as tpsum,
            tc.tile_pool(name="tb", bufs=8) as tp,
            tc.tile_pool(name="jk", bufs=1) as jp,
            tc.tile_pool(name="al", bufs=2) as alp,
            tc.tile_pool(name="sm", bufs=1) as smp,
            tc.tile_pool(name="ms", bufs=1) as ms,
        ):
            # ---------------- prep: constants & weights ----------------
            eye = const.tile([P, P], f32, tag="eye")
            nc.sync.dma_start(eye[:], eye_dram[:, :])
            eye_bf = const.tile([P, P], bf16, tag="eye_bf")
            nc.vector.tensor_copy(eye_bf[:], eye[:])

            # small prep loads first so they never queue behind feature loads
            h_nat = ms.tile([X, DEC], f32, tag="h_nat")
            nc.sync.dma_start(h_nat[:], hid[:, :])
            b1_nat = ms.tile([1, ATT], f32, tag="b1_nat")
            nc.sync.dma_start(b1_nat[:], w1b[None, :])
            b2_nat = ms.tile([1, ATT], f32, tag="b2_nat")
            nc.sync.dma_start(b2_nat[:], w2b[None, :])
            v_nat = ms.tile([1, ATT], f32, tag="v_nat")
            nc.sync.dma_start(v_nat[:], vw[None, :])

            # W1 fp8 DoubleRow layout (host-prepacked), 4 queue-spread DMAs
            w1q = const.tile([P, NE * ATT], fp8, tag="w1q")
            for q in range(4):
                nc.sync.dma_start(
                    w1q[:, 2 * ATT * q : 2 * ATT * (q + 1)],
                    w1q_d[:, 2 * ATT * q : 2 * ATT * (q + 1)],
                )
            w1v = w1q.rearrange("p (c i a) -> p c i a", c=NKP, i=2)

            # W2 bf16 (host-prepacked)
            w2t = []
            for e in range(ND):
                t = const.tile([P, ATT], bf16, tag=f"w2_{e}")
                nc.sync.dma_start(t[:], w2[P * e : P * (e + 1), :])
                w2t.append(t)

            # ---------------- per-example staging ----------------
            # fT bf16 [p, ec*L + l] = features[x, l, 128ec + p] via XBAR
            # transposed loads; DVE casts each k-pair slab to fp8 (emitted
            # separately, off the latency-critical DVE stretch).
            def emit_slab(x, ftb, ec):
                # whole-slab XBAR transpose: [1024 l, 128 e] -> [128, 1024]
                nc.sync.dma_start(
                    ftb[:, L * ec : L * (ec + 1)],
                    feats[x, :, P * ec : P * (ec + 1)],
                    transpose=True,
                )

            def emit_load_ft(x, defer=False):
                ftb = ftbp.tile([P, NE * L], bf16, tag="ftb", name=f"ftb{x}")
                ft8 = ftp.tile([P, NE * L], fp8, tag="ft8", name=f"ft8{x}")
                if defer:
                    slabs = [
                        (lambda x=x, ftb=ftb, ec=ec: emit_slab(x, ftb, ec))
                        for ec in range(NE)
                    ]
                else:
                    slabs = []
                    for ec in range(NE):
                        emit_slab(x, ftb, ec)
                return ftb, ft8, slabs

            def emit_casts_ft(ftb, ft8):
                for c in range(NKP):
                    nc.vector.tensor_copy(
                        ft8[:, 2 * L * c : 2 * L * (c + 1)],
                        ftb[:, 2 * L * c : 2 * L * (c + 1)],
                    )

            # hT_all[p, c, x] = hid[x, 128c + p] via natural load + PE transpose
            hn_bf = ms.tile([X, DEC], bf16, tag="hn_bf")
            nc.vector.tensor_copy(hn_bf[:], h_nat[:])
            hTb = ms.tile([P, ND, X], bf16, tag="hTb")
            for c in range(ND):
                tps_h = tpsum.tile([P, X], bf16, tag="tp", name=f"tpsh{c}")
                nc.tensor.transpose(tps_h[:], hn_bf[:, P * c : P * (c + 1)], eye_bf[0:X, 0:X])
                nc.vector.tensor_copy(hTb[:, c, :], tps_h[:])

            # bias vectors: natural load, PE-transpose each into [128, NA]
            def load_transposed_vec(nat, name, dt, scale=None):
                tps_v = tpsum.tile([P, NA], f32, tag="tp", name=f"tps_{name}")
                for c in range(NA):
                    nc.tensor.transpose(
                        tps_v[:, c : c + 1], nat[:, P * c : P * (c + 1)], eye[0:1, 0:1]
                    )
                dst = ms.tile([P, NA], dt, tag=name, name=name)
                if scale is None:
                    nc.vector.tensor_copy(dst[:], tps_v[:])
                else:
                    nc.vector.tensor_scalar_mul(dst[:], tps_v[:], scale)
                return dst

            b1T = load_transposed_vec(b1_nat, "b1T", f32)
            b2T = load_transposed_vec(b2_nat, "b2T", f32)
            # V transposed, scaled x16 in fp8, padded to 8 columns (dual-fp8
            # ldweights rejects single-column stationaries)
            VPAD = 128
            tps_vw = tpsum.tile([P, NA], f32, tag="tp", name="tps_vw")
            for c in range(NA):
                nc.tensor.transpose(
                    tps_vw[:, c : c + 1], v_nat[:, P * c : P * (c + 1)], eye[0:1, 0:1]
                )
            v8d = ms.tile([P, NA * VPAD], fp8, tag="v8d")
            v8v = v8d.rearrange("p (a m) -> p a m", a=NA)
            nc.vector.memset(v8d[:], 0.0)
            nc.vector.tensor_scalar_mul(v8v[:, :, 0], tps_vw[:], W1S)
            bT = ms.tile([P, NA], f32, tag="bT")
            nc.vector.tensor_add(bT[:], b1T[:], b2T[:])

            # proj_h transposed, plus bias: phb[p, a, x]
            phb = ms.tile([P, NA, X], f32, tag="phb")
            for a in range(NA):
                ph_ps = psum.tile([P, X], f32, tag="mm", name=f"phps{a}")
                for e in range(ND):
                    nc.tensor.matmul(
                        ph_ps[:],
                        w2t[e][:, P * a : P * (a + 1)],
                        hTb[:, e, :],
                        start=(e == 0),
                        stop=(e == ND - 1),
                    )
                nc.vector.tensor_scalar_add(phb[:, a, :], ph_ps[:], bT[:, a : a + 1])

            ones_bf = const.tile([1, P], bf16, tag="ones_bf")
            nc.vector.memset(ones_bf[:], 1.0)

            # feature staging for the first two examples (after the prep
            # section so the prep's DVE/PE chains aren't queued behind the
            # big casts)
            ftb_map = {}
            ft_map = {}
            ftb_map[0], ft_map[0], _ = emit_load_ft(0)
            ftb_map[1], ft_map[1], _ = emit_load_ft(1)
            emit_casts_ft(ftb_map[0], ft_map[0])

            # ---------------- main per-example pipeline ----------------
            # V-dot matmuls trail the tanh by one block so the PE never
            # waits on ACT.
            pending = []

            def flush_pending():
                for sc_ap, vw_ap, tb_ap, st, sp in pending:
                    nc.tensor.matmul(
                        sc_ap, vw_ap, tb_ap, start=st, stop=sp, perf_mode=DR
                    )
                pending.clear()

            def emit_softmax(x, sc_pair):
                # scores are bounded (|s| < ~3), so exp() cannot overflow:
                # skip the max-subtraction and exponentiate the PSUM halves
                # directly.  Normalization is folded into the context scale
                # and a deferred alpha store.  Scores arrive x16 (V
                # pre-scaled x16 for fp8) - undone via the exp input scale.
                ebf = alp.tile([1, L], bf16, tag="ebf")
                ss0 = alp.tile([1, 1], f32, tag="ss0")
                ss1 = alp.tile([1, 1], f32, tag="ss1")
                nc.scalar.activation(
                    ebf[:, 0:LH], sc_pair[0][0:1, :], Exp, scale=1.0 / W1S, accum_out=ss0[:]
                )
                nc.scalar.activation(
                    ebf[:, LH:L], sc_pair[1][0:1, :], Exp, scale=1.0 / W1S, accum_out=ss1[:]
                )
                ssum = alp.tile([1, 1], f32, tag="ssum")
                nc.vector.tensor_add(ssum[:], ss0[:], ss1[:])
                rinv = alp.tile([1, 1], f32, tag="rinv")
                nc.vector.reciprocal(rinv[:], ssum[:])
                return ebf, ebf, rinv

            # context accumulation on DVE over unnormalized exp weights
            # (emitted right after softmax).  For the last example the
            # broadcast runs on the (then idle) PE into PSUM instead of the
            # slower Pool engine, shortening the drain tail.
            def emit_ctx_accum(x, ebf, last=False):
                ftb = ftb_map[x]
                ctx_x = alp.tile([P, NE], f32, tag="ctx_x")
                if last:
                    ctx_b = alp.tile([P, NE], f32, tag="ctx_b")
                    erep = [
                        psum.tile([P, LH], f32, tag="mm", name=f"erep{h}")
                        for h in range(NL)
                    ]
                    for h in range(NL):
                        nc.tensor.matmul(
                            erep[h][:], ones_bf[:], ebf[:, LH * h : LH * (h + 1)]
                        )
                    for e in range(NE):
                        for h in range(NL):
                            jk = jp.tile([P, LH], fp8, tag="jk2", name=f"jk{e}_{h}")
                            nc.vector.scalar_tensor_tensor(
                                out=jk[:],
                                in0=ftb[:, e * L + LH * h : e * L + LH * (h + 1)],
                                scalar=1.0,
                                in1=erep[h][:],
                                op0=mult,
                                op1=mult,
                                accum_out=(ctx_x if h == 0 else ctx_b)[:, e : e + 1],
                            )
                    nc.vector.tensor_add(ctx_x[:], ctx_x[:], ctx_b[:])
                    return ctx_x
                arep = alp.tile([P, L], bf16, tag="arep")
                nc.gpsimd.partition_broadcast(arep[:], ebf[:])
                for e in range(NE):
                    jk = jp.tile([P, L], fp8, tag="jk")
                    nc.vector.scalar_tensor_tensor(
                        out=jk[:],
                        in0=ftb[:, e * L : (e + 1) * L],
                        scalar=1.0,
                        in1=arep[:],
                        op0=mult,
                        op1=mult,
                        accum_out=ctx_x[:, e : e + 1],
                    )
                return ctx_x

            Copy = mybir.ActivationFunctionType.Copy

            def mk_alpha_store(x, esb, rinv):
                def t_store():
                    a32 = alp.tile([1, L], f32, tag="a32")
                    nc.scalar.activation(a32[:], esb[:], Copy, scale=rinv[:])
                    nc.gpsimd.dma_start(alpha_o[x, :], a32[:])

                return t_store

            def mk_ctx_store(x, ctx_x, rinv):
                def t_store():
                    rrep = alp.tile([P, 1], f32, tag="rrep", name=f"rrep{x}")
                    nc.gpsimd.partition_broadcast(rrep[:], rinv[:])
                    cxs = alp.tile([P, NE], f32, tag="cxs", name=f"cxs{x}")
                    nc.vector.tensor_scalar_mul(cxs[:], ctx_x[:], rrep[:])
                    ct_ps = tpsum.tile([NE, P], f32, tag="tp", name=f"ctps{x}")
                    nc.tensor.transpose(ct_ps[:], cxs[:], eye[:])
                    ctr = alp.tile([NE, P], f32, tag="ctr")
                    nc.scalar.activation(ctr[:], ct_ps[:], Copy)
                    nc.gpsimd.dma_start(
                        ctx_o.rearrange("x (e c) -> x e c", e=NE)[x], ctr[:]
                    )
                    ftb_map.pop(x, None)

                return t_store

            def emit_epilogue(x, sc_pair, last=False):
                esb, ebf, rinv = emit_softmax(x, sc_pair)
                ctx_x = emit_ctx_accum(x, ebf, last=last)
                return (
                    [mk_alpha_store(x, esb, rinv)],
                    [mk_ctx_store(x, ctx_x, rinv)],
                )

            tasks = []
            late_tasks = []
            epi = None  # previous example's epilogue, emitted after its flush
            for x in range(X):
                slabs = []
                if x + 2 < X:
                    ftb_map[x + 2], ft_map[x + 2], slabs = emit_load_ft(
                        x + 2, defer=True
                    )
                ftv = ft_map[x].rearrange("p (c i l) -> p c i l", c=NKP, i=2)

                sc_h = {
                    0: spsum.tile([VPAD, LH], f32, tag="sc", name=f"sch0_{x}"),
                    1: spsum.tile([VPAD, LH], f32, tag="sc", name=f"sch1_{x}"),
                }
                for a in range(NA):
                    # issue x+2's transposed loads 2-per-block in the first
                    # half of the window: spreads SP-sequencer cost but still
                    # leaves a full window of transfer time before the casts
                    for _ in range(2):
                        if slabs:
                            slabs.pop(0)()
                    pp = [
                        psum.tile([P, LH], f32, tag="mm", name=f"pp{x}_{a}_{i}")
                        for i in range(NL)
                    ]
                    for c in range(NKP):
                        for lh in range(NL):
                            nc.tensor.matmul(
                                pp[lh][:],
                                w1v[:, c, :, P * a : P * (a + 1)],
                                ftv[:, c, :, LH * lh : LH * (lh + 1)],
                                start=(c == 0),
                                stop=(c == NKP - 1),
                                perf_mode=DR,
                            )
                        if c == 2:
                            flush_pending()
                            if a == 0 and epi is not None:
                                t5, t7 = epi()
                                tasks += t5
                                late_tasks += t7
                                epi = None
                            if a == 1 and x + 1 < X:
                                emit_casts_ft(ftb_map[x + 1], ft_map[x + 1])
                            if a == 2:
                                # ct-store of example x-2: its DVE chain had a
                                # full extra window to finish
                                while len(late_tasks) > 1:
                                    late_tasks.pop(0)()
                            if a >= 5:
                                while tasks:
                                    tasks.pop(0)()
                    # tanh pairs in fp8: apair q = a//2, slot i = a%2; one
                    # DoubleRow V-dot per (q, lh) once the pair completes.
                    if a % 2 == 0:
                        tb8 = [
                            tp.tile([P, 2 * LH], fp8, tag="tb", name=f"tb{x}_{a}_{i}")
                            for i in range(NL)
                        ]
                    for lh in range(NL):
                        nc.scalar.activation(
                            tb8[lh][:, LH * (a % 2) : LH * (a % 2 + 1)],
                            pp[lh][:],
                            Tanh,
                            bias=phb[:, a, x : x + 1],
                            scale=1.0 / W1S,
                        )
                        if a % 2 == 1:
                            q = a // 2
                            pending.append(
                                (
                                    sc_h[lh][:],
                                    v8v[:, 2 * q : 2 * q + 2, :],
                                    tb8[lh].rearrange("p (i l) -> p i l", i=2),
                                    q == 0,
                                    q == NKP - 1,
                                )
                            )

                epi = lambda x=x, sc=sc_h: emit_epilogue(x, sc, last=(x == X - 1))

            # tail: pending ct-stores, then last example's epilogue inline
            flush_pending()
            t5, t7 = epi()
            for t in late_tasks + t5 + t7:
                t()

    nc.compile()
    return nc


def _prepack(W1_w):
    import ml_dtypes

    # w1q[p, ec*ATT + a] = fp8e4(16 * W1[128*ec + p, a])
    w = (np.asarray(W1_w, dtype=np.float32) * W1S).reshape(NE, P, ATT)
    w = np.ascontiguousarray(w.transpose(1, 0, 2).reshape(P, NE * ATT))
    return w.astype(ml_dtypes.float8_e4m3fn)


def kernel(features, hidden_state, W1_w, W1_b, W2_w, W2_b, V_w, V_b):
    import ml_dtypes
    from concourse.bass_utils import run_bass_kernel_spmd

    if "nc" not in _CACHE:
        _CACHE["nc"] = _build()
    nc = _CACHE["nc"]

    features = np.asarray(features, dtype=np.float32).astype(ml_dtypes.bfloat16)
    hidden_state = np.ascontiguousarray(np.asarray(hidden_state, dtype=np.float32))
    w1q = _prepack(W1_w)
    W1_b = np.ascontiguousarray(np.asarray(W1_b, dtype=np.float32))
    W2_w = np.asarray(W2_w, dtype=np.float32).astype(ml_dtypes.bfloat16)
    W2_b = np.ascontiguousarray(np.asarray(W2_b, dtype=np.float32))
    V_w = np.ascontiguousarray(np.asarray(V_w, dtype=np.float32))

    in_maps = []
    for c in range(N_CORES):
        in_maps.append(
            {
                "features": np.ascontiguousarray(features[c * X : (c + 1) * X]),
                "hidden_state": np.ascontiguousarray(hidden_state[c * X : (c + 1) * X]),
                "W1_q": w1q,
                "W1_b": W1_b,
                "W2_w": W2_w,
                "W2_b": W2_b,
                "V_w": V_w,
            }
        )

    res = run_bass_kernel_spmd(nc, in_maps, list(range(N_CORES)), **_CACHE.get("run_kwargs", {}))
    _CACHE["last_result"] = res
    alpha = np.concatenate([res.results[c]["alpha"] for c in range(N_CORES)], axis=0)
    context = np.concatenate([res.results[c]["context"] for c in range(N_CORES)], axis=0)
    return alpha, context
